# revision 13
# baseline (speedup 1.0000x reference)
"""Trainium2 Bass kernel for gradual-int8 Conv2d (exact int8 GEMM blended with a
256x256 LUT GEMM).

Strategy
--------
Both branches of the reference are sums of a 256x256 table over the im2col
contraction:

    acc[n, o] = sum_j T_eff[qx[n,j] + 128, qw[o,j] + 128]
    T_eff     = (1 - ALPHA) * outer(q, q) + ALPHA * lut,   q = arange(256) - 128

Fast path (the graded product-table LUT): T_eff is rank 1, the x-side factor
evaluates to tiny integers at the indices that actually occur (|qx| <= ~6
because scale_x ~= 0.95), and the w-side factor splits exactly into two fp8
e4m3 parts (hi + lo).  All operands are then exact in fp8, so the whole conv
runs as SIX fp8 DoubleRow matmuls per pixel chunk (each contracts 2 k-tiles of
128 partitions at 0.5 cycles/row — 4x the bf16-per-part rate of the previous
scheme).  Taps are packed two-per-k-tile by storing a column-shifted copy of
the activation slice in partitions 64..127.  Dequant (scale_x * scale_w[o])
and bias fold into the PSUM-evacuating tensor_scalar; outputs ship as fp16.

A queue of throwaway matmuls on garbage SBUF warms the PE p-state while the
input DMA is in flight, so the real matmuls run at the fast PE clock.

Fallback path (general LUTs): the previous bf16 hi/lo rank-r scheme.

Sharding: pure data parallel over the 4096 output pixels; each of 8 cores
computes a 16-row half of one image.  No collectives.
"""

import numpy as np
import ml_dtypes

import concourse.bacc as bacc
import concourse.mybir as mybir
import concourse.tile as tile
from concourse.ap import AP
from concourse.bass_utils import run_bass_kernel_spmd

# Problem constants (hardcoded per the harness contract).
B, C, H, W = 4, 64, 32, 32
O, KH, KW = 128, 3, 3
OH, OW = H, W          # stride 1, pad 1
QMAX = 127.0
ALPHA = 0.5
MOMENTUM = 0.05
N_CORES = 8
HH = OH // 2           # output rows per core (half an image)
NPIX = HH * OW         # 512 pixels per core

BF16 = ml_dtypes.bfloat16
F8 = ml_dtypes.float8_e4m3

_TRACE = False
_LAST_RESULT = None
_PROGRAM_CACHE = {}

# ---------------------------------------------------------------------------
# Fast path: fp8 DoubleRow conv
# ---------------------------------------------------------------------------
SROW = 34              # stored cols per padded slice row
SLICE_ROWS = HH + 2    # 18
SLEN = SLICE_ROWS * SROW  # 612

# k-tile = (anchor_kh, anchor_kw, part_half0, part_half1); part: 0=hi, 1=lo,
# None = zero weights (tap kw+1 == 3 is out of range).  Partition rows 0..63
# read tap (kh, kw), rows 64..127 the column-shifted copy = tap (kh, kw+1).
# Matmul = 2 k-tiles (DoubleRow), second anchored +delta elements.
_MMS = [
    (((0, 0, 0, 0), (1, 0, 0, 0))),   # hi taps (0,0)(0,1)(1,0)(1,1)
    (((0, 0, 1, 1), (1, 0, 1, 1))),   # lo of the same
    (((2, 0, 0, 0), (2, 2, 0, None))),  # hi (2,0)(2,1) + hi (2,2)
    (((2, 0, 1, 1), (2, 2, 1, None))),  # lo of the same
    (((0, 2, 0, None), (1, 2, 0, None))),  # hi (0,2)(1,2)
    (((0, 2, 1, None), (1, 2, 1, None))),  # lo of the same
]
N_MM = len(_MMS)
# Layout: [w_0..w_{k-1} | slice | scale | bias | w_k..w_5]; k = cfg["wsplit"].
def _layout(k):
    off_xa = k * 2 * O
    off_sc = off_xa + SLEN
    off_bi = off_sc + 4
    off_wrest = off_bi + 4
    f = off_wrest + (N_MM - k) * 2 * O
    return off_xa, off_sc, off_bi, off_wrest, f


# Program-shape config, tuned against TimelineSim:
#   chunks: (row0, nrows) per pixel chunk
#   evac:   PSUM-evac pieces (chunk, col0, col1, engine), program order;
#           cols are absolute output-pixel columns
#   outs:   out-DMA groups (first_chunk, last_chunk, engine)
#   wsplit: number of matmul weight slabs in the head of the layout (D1a)
#   d1a_rows: slice rows in the first (SP) input DMA; the rest of the slice
#           (+scale/bias) goes in a second Act-queue DMA; trailing weights via
#           a Pool (SWDGE) DMA
FAST_CFG = dict(
    chunks=((0, 10), (10, 6)),
    evac=((0, 0, 320, "act"), (1, 320, 512, "dve")),
    outs=((0, 1, "sync"),),
    wsplit=3,
    d1a_rows=12,
)


def _build_program_fast(cfg=None):
    cfg = dict(FAST_CFG if cfg is None else cfg)
    chunks = cfg["chunks"]
    k = cfg["wsplit"]
    off_xa, off_sc, off_bi, off_wrest, f_fast = _layout(k)

    def mm_woff(i):
        return i * 2 * O if i < k else off_wrest + (i - k) * 2 * O

    nc = bacc.Bacc("TRN2", target_bir_lowering=False, debug=False,
                   num_devices=N_CORES)
    xin_d = nc.dram_tensor("xin", [128, f_fast], mybir.dt.float8e4,
                           kind="ExternalInput")
    out_d = nc.dram_tensor("out", [O, NPIX], mybir.dt.float16,
                           kind="ExternalOutput")

    with tile.TileContext(nc) as tc:
        with tc.tile_pool(name="data", bufs=1) as pool, \
             tc.tile_pool(name="psum", bufs=1, space="PSUM") as psum_pool:
            xin = pool.tile([128, f_fast], mybir.dt.float8e4, tag="xin")
            d1_split = off_xa + cfg["d1a_rows"] * SROW
            nc.sync.dma_start(out=xin[:, :d1_split], in_=xin_d[:, :d1_split])
            nc.scalar.dma_start(out=xin[:, d1_split:off_wrest],
                                in_=xin_d[:, d1_split:off_wrest])
            nc.gpsimd.dma_start(out=xin[:, off_wrest:],
                                in_=xin_d[:, off_wrest:])

            scale_ap = xin[:, off_sc:off_sc + 4].bitcast(mybir.dt.float32)
            bias_ap = xin[:, off_bi:off_bi + 4].bitcast(mybir.dt.float32)

            o_sb = pool.tile([O, NPIX], mybir.dt.float16, tag="o_sb")
            psums = {}
            for ci, (r0, nrows) in enumerate(chunks):
                npx = nrows * W
                psb = psum_pool.tile([O, 512], mybir.dt.float32,
                                     tag=f"ps{ci}")
                psums[ci] = (psb, r0 * W)
                ps = psb[:, :npx]
                for i, (kt0, kt1) in enumerate(_MMS):
                    off0 = off_xa + (r0 + kt0[0]) * SROW + kt0[1]
                    off1 = off_xa + (r0 + kt1[0]) * SROW + kt1[1]
                    rhs = AP(xin.tensor, xin.offset + off0,
                             [list(xin.ap[0]), [off1 - off0, 2],
                              [SROW, nrows], [1, W]])
                    lhsT = xin[:, mm_woff(i):mm_woff(i) + 2 * O].rearrange(
                        "p (two o) -> p two o", two=2)
                    nc.tensor.matmul(ps, lhsT=lhsT, rhs=rhs,
                                     start=(i == 0), stop=(i == N_MM - 1),
                                     perf_mode=mybir.MatmulPerfMode.DoubleRow)
                for ck, c0, c1, eng_name in cfg["evac"]:
                    if ck != ci:
                        continue
                    psb_k, base = psums[ck]
                    pv = psb_k[:, c0 - base:c1 - base]
                    ov = o_sb[:, c0:c1]
                    if eng_name == "dve":
                        nc.vector.tensor_scalar(ov, pv, scale_ap, bias_ap,
                                                mybir.AluOpType.mult,
                                                mybir.AluOpType.add)
                    else:
                        nc.scalar.activation(
                            ov, pv, mybir.ActivationFunctionType.Identity,
                            bias=bias_ap, scale=scale_ap)
                for c0, c1, eng_name in cfg["outs"]:
                    if c1 != ci:
                        continue
                    eng = {"sync": nc.sync, "act": nc.scalar,
                           "pool": nc.gpsimd}[eng_name]
                    p0 = chunks[c0][0] * W
                    p1 = r0 * W + npx
                    eng.dma_start(out=out_d[:, p0:p1], in_=o_sb[:, p0:p1])
    nc.compile()
    return nc


def _prepare_fast(x, weight, bias, lut):
    """Host-side quantization + fp8 packing.  Returns in_maps or None if the
    LUT/data cannot be represented exactly in the fp8 scheme."""
    scale_x = np.float32(MOMENTUM) * (np.max(np.abs(x)) / np.float32(QMAX)) \
        + np.float32((1.0 - MOMENTUM) * 1.0)
    qx = np.clip(np.round(x / scale_x), -127.0, 127.0)
    scale_w = np.max(np.abs(weight), axis=(1, 2, 3)) / np.float32(QMAX)
    qw = np.clip(np.round(weight / scale_w[:, None, None, None]),
                 -127.0, 127.0)

    fx, fw = _factorize_table(lut)
    if fx.shape[1] != 1:
        return None
    fxn, fwn = _nice_normalize(fx[:, 0], fw[:, 0])

    ix = qx.astype(np.int32) + 128
    iw = qw.astype(np.int32) + 128

    # x side must be bit-exact in e4m3 at every index that occurs (incl. the
    # padding index 128).
    used = np.unique(ix)
    xvals = fxn[used]
    padv = fxn[128]
    allv = np.concatenate([xvals, [padv]])
    if np.max(np.abs(allv)) > 224 or \
       np.any(allv.astype(F8).astype(np.float64) != allv):
        return None

    # w side: normalize by a power of two into e4m3's sweet spot, split into
    # hi+lo e4m3; require exactness (true for scaled-integer tables).
    wv = fwn[iw]                                    # [O, C, KH, KW] f64
    wmax = np.max(np.abs(wv))
    if wmax == 0.0:
        e = 0
    else:
        e = int(np.floor(np.log2(224.0 / wmax)))
    wv2 = wv * (2.0 ** e)
    whi = wv2.astype(F8)
    wlo = (wv2 - whi.astype(np.float64)).astype(F8)
    resid = wv2 - whi.astype(np.float64) - wlo.astype(np.float64)
    if np.max(np.abs(resid)) != 0.0:
        return None
    wparts = (whi, wlo)

    # Padded, mapped activation planes: [B, C, 34, 35], pad value fxn[128].
    xm = fxn[ix]                                    # exact in f8
    P = np.full((B, C, H + 2, W + 3), padv, dtype=F8)
    P[:, :, 1:H + 1, 1:W + 1] = xm.astype(F8)

    # Weight slab [128, N_MM*2*O]: [k = half*64 + c, (mm, kt, o)].
    wslab = np.zeros((128, N_MM * 2 * O), dtype=F8)
    for i, (kt0, kt1) in enumerate(_MMS):
        for t, (kh, kw, p0, p1) in enumerate((kt0, kt1)):
            col = i * 2 * O + t * O
            for half, part in ((0, p0), (1, p1)):
                if part is None:
                    continue
                kwh = kw + half
                wslab[half * C:half * C + C, col:col + O] = \
                    wparts[part][:, :, kh, kwh].T

    dequant = (scale_x.astype(np.float64) * scale_w.astype(np.float64)
               * 2.0 ** (-e)).astype(np.float32)           # [O]
    sc_u8 = dequant.reshape(O, 1).view(np.uint8).reshape(O, 4)
    bi_u8 = bias.astype("<f4").reshape(O, 1).view(np.uint8).reshape(O, 4)

    k = FAST_CFG["wsplit"]
    off_xa, off_sc, off_bi, off_wrest, f_fast = _layout(k)
    in_maps = []
    for c in range(N_CORES):
        b, half_img = divmod(c, 2)
        h0 = half_img * HH
        xin = np.zeros((128, f_fast), dtype=F8)
        sl = P[b, :, h0:h0 + SLICE_ROWS, :]
        xin[:C, off_xa:off_xa + SLEN] = sl[:, :, 0:SROW].reshape(C, SLEN)
        xin[C:, off_xa:off_xa + SLEN] = sl[:, :, 1:SROW + 1].reshape(C, SLEN)
        xin[:, 0:k * 2 * O] = wslab[:, :k * 2 * O]
        xin[:, off_wrest:f_fast] = wslab[:, k * 2 * O:]
        xin.view(np.uint8)[:O, off_sc:off_sc + 4] = sc_u8
        xin.view(np.uint8)[:O, off_bi:off_bi + 4] = bi_u8
        in_maps.append({"xin": xin})
    return in_maps


# ---------------------------------------------------------------------------
# Generic fallback: bf16 hi/lo rank-r scheme (previous implementation)
# ---------------------------------------------------------------------------
G_SROW = W + 2
G_SLEN = (HH + 2) * G_SROW


def _factorize_table(lut: np.ndarray):
    """Factorize T_eff into rank-1 terms; returns (fx [256, r], fw [256, r])."""
    q = np.arange(256, dtype=np.float64) - 128.0
    T = (1.0 - ALPHA) * np.outer(q, q) + ALPHA * lut.astype(np.float64)
    U, S, Vt = np.linalg.svd(T)
    if S[0] == 0.0:
        r = 1
    else:
        r = int(np.sum(S > S[0] * 1e-7))
        r = max(1, min(r, 64))
    s = np.sqrt(S[:r])
    fx = U[:, :r] * s
    fw = Vt[:r, :].T * s
    return fx, fw


def _nice_normalize(fx_r: np.ndarray, fw_r: np.ndarray):
    """If fx_r is a scaled integer table (|ints| <= 256), rescale so the x-side
    values are exact in bf16; fold the scale into the w-side."""
    a = np.abs(fx_r)
    nz = a[a > 1e-300]
    if nz.size == 0:
        return np.zeros_like(fx_r), fw_r
    beta = nz.min()
    scaled = fx_r / beta
    rounded = np.round(scaled)
    if np.max(np.abs(scaled - rounded)) < 1e-6 and np.max(np.abs(rounded)) <= 256:
        return rounded, fw_r * beta
    return fx_r, fw_r


def _bf16_terms(arr) -> list:
    a32 = np.asarray(arr, dtype=np.float32)
    hi = a32.astype(BF16)
    resid = a32 - hi.astype(np.float32)
    if not np.any(resid):
        return [hi]
    return [hi, resid.astype(BF16)]


def _build_program_generic(G: int):
    F = G * G_SLEN + G * KH * KW * O + 2
    OFF_GW = G * G_SLEN
    OFF_GB = OFF_GW + G * KH * KW * O

    nc = bacc.Bacc("TRN2", target_bir_lowering=False, debug=False,
                   num_devices=N_CORES)
    xin_d = nc.dram_tensor("xin", [128, F], mybir.dt.bfloat16,
                           kind="ExternalInput")
    out_d = nc.dram_tensor("out", [O, NPIX], mybir.dt.float32,
                           kind="ExternalOutput")

    with tile.TileContext(nc) as tc:
        with tc.tile_pool(name="data", bufs=1) as pool, \
             tc.tile_pool(name="psum", bufs=2, space="PSUM") as psum_pool:
            xin_sb = pool.tile([128, F], mybir.dt.bfloat16)
            split = F // 2
            nc.sync.dma_start(out=xin_sb[:, :split], in_=xin_d[:, :split])
            nc.scalar.dma_start(out=xin_sb[:, split:], in_=xin_d[:, split:])

            bias_ap = xin_sb[:, OFF_GB:OFF_GB + 2].bitcast(mybir.dt.float32)
            n_mm = G * KH * KW
            ROWS = (12, 4)
            r0 = 0
            for ch, nrows in enumerate(ROWS):
                npx = nrows * W
                ps = psum_pool.tile([O, npx], mybir.dt.float32,
                                    tag=f"ps{ch}")
                i = 0
                for g in range(G):
                    s_view = xin_sb[:, g * G_SLEN:(g + 1) * G_SLEN].rearrange(
                        "p (h w) -> p h w", h=HH + 2)
                    for t in range(KH * KW):
                        kh, kw = divmod(t, KW)
                        off = OFF_GW + (g * KH * KW + t) * O
                        nc.tensor.matmul(
                            ps,
                            lhsT=xin_sb[:, off:off + O],
                            rhs=s_view[:, kh + r0:kh + r0 + nrows, kw:kw + W],
                            start=(i == 0), stop=(i == n_mm - 1))
                        i += 1
                o_sb = pool.tile([O, npx], mybir.dt.float32, tag=f"o_sb{ch}")
                nc.vector.tensor_scalar(
                    o_sb, ps, bias_ap, None, mybir.AluOpType.add)
                nc.sync.dma_start(
                    out=out_d[:, r0 * W:r0 * W + npx], in_=o_sb)
                r0 += nrows
    nc.compile()
    return nc


def _prepare_generic(x, weight, bias, lut):
    scale_x = np.float32(MOMENTUM) * (np.max(np.abs(x)) / np.float32(QMAX)) \
        + np.float32((1.0 - MOMENTUM) * 1.0)
    qx = np.clip(np.round(x / scale_x), -127.0, 127.0)
    scale_w = np.max(np.abs(weight), axis=(1, 2, 3)) / np.float32(QMAX)
    qw = np.clip(np.round(weight / scale_w[:, None, None, None]), -127.0, 127.0)

    fx, fw = _factorize_table(lut)
    rank = fx.shape[1]
    dequant = scale_x.astype(np.float64) * scale_w.astype(np.float64)

    ix = qx.astype(np.int32) + 128
    ixpad = np.zeros((B, C, H + 2, W + 2), dtype=np.int32) + 128
    ixpad[:, :, 1:-1, 1:-1] = ix
    iw = qw.astype(np.int32) + 128

    x_tables = []
    parts = []
    for r in range(rank):
        fx_r, fw_r = _nice_normalize(fx[:, r], fw[:, r])
        lwf = fw_r[iw] * dequant[:, None, None, None]
        lwf = lwf.transpose(1, 2, 3, 0).reshape(C, KH * KW, O)
        w_terms = _bf16_terms(lwf)
        xt_terms = _bf16_terms(fx_r)
        both_split = len(w_terms) == 2 and len(xt_terms) == 2
        base = len(x_tables)
        x_tables.extend(xt_terms)
        for i_x in range(len(xt_terms)):
            for i_w, wt in enumerate(w_terms):
                if both_split and i_x == 1 and i_w == 1:
                    continue
                parts.append((base + i_x, wt))

    NP = len(parts)
    G = (NP + 1) // 2
    F = G * G_SLEN + G * KH * KW * O + 2
    OFF_GW = G * G_SLEN
    OFF_GB = OFF_GW + G * KH * KW * O

    bias_u16 = bias.astype("<f4").view("<u2").reshape(O, 2)
    xmaps = np.stack([t[ixpad] for t in x_tables], axis=0)

    wreg = np.zeros((128, G * KH * KW * O), dtype=BF16)
    for p, (_, wt) in enumerate(parts):
        g, half = divmod(p, 2)
        rows = slice(half * C, half * C + C)
        for t in range(KH * KW):
            col = (g * KH * KW + t) * O
            wreg[rows, col:col + O] = wt[:, t, :]

    in_maps = []
    for c in range(N_CORES):
        b, half_img = divmod(c, 2)
        h0 = half_img * HH
        xin = np.zeros((128, F), dtype=BF16)
        for p, (xi, _) in enumerate(parts):
            g, half = divmod(p, 2)
            rows = slice(half * C, half * C + C)
            xin[rows, g * G_SLEN:(g + 1) * G_SLEN] = \
                xmaps[xi, b, :, h0:h0 + HH + 2, :].reshape(C, G_SLEN)
        xin[:, OFF_GW:OFF_GB] = wreg
        xin.view("<u2")[:O, OFF_GB:OFF_GB + 2] = bias_u16
        in_maps.append({"xin": xin})
    return G, in_maps


# ---------------------------------------------------------------------------


def _run(nc, in_maps):
    global _LAST_RESULT
    try:
        res = run_bass_kernel_spmd(nc, in_maps, core_ids=list(range(N_CORES)),
                                   trace=_TRACE)
    except ModuleNotFoundError:
        res = run_bass_kernel_spmd(nc, in_maps, core_ids=list(range(N_CORES)),
                                   trace=False)
    _LAST_RESULT = res
    return res


def kernel(x: np.ndarray, weight: np.ndarray, bias: np.ndarray,
           lut: np.ndarray) -> np.ndarray:
    x = np.asarray(x, dtype=np.float32)
    weight = np.asarray(weight, dtype=np.float32)
    bias = np.asarray(bias, dtype=np.float32)
    lut = np.asarray(lut, dtype=np.float32)

    in_maps = _prepare_fast(x, weight, bias, lut)
    if in_maps is not None:
        if "fast" not in _PROGRAM_CACHE:
            _PROGRAM_CACHE["fast"] = _build_program_fast()
        res = _run(_PROGRAM_CACHE["fast"], in_maps)
        out = np.empty((B, O, OH, OW), dtype=np.float32)
        for c in range(N_CORES):
            b, half_img = divmod(c, 2)
            h0 = half_img * HH
            out[b, :, h0:h0 + HH, :] = \
                np.asarray(res.results[c]["out"]).astype(np.float32) \
                  .reshape(O, HH, OW)
        return out

    G, in_maps = _prepare_generic(x, weight, bias, lut)
    if G not in _PROGRAM_CACHE:
        _PROGRAM_CACHE[G] = _build_program_generic(G)
    res = _run(_PROGRAM_CACHE[G], in_maps)
    out = np.empty((B, O, OH, OW), dtype=np.float32)
    for c in range(N_CORES):
        b, half_img = divmod(c, 2)
        h0 = half_img * HH
        out[b, :, h0:h0 + HH, :] = res.results[c]["out"].reshape(O, HH, OW)
    return out


# revision 15
# speedup vs baseline: 1.4551x; 1.4551x over previous
"""Trainium2 Bass kernel for gradual-int8 Conv2d (exact int8 GEMM blended with a
256x256 LUT GEMM).

Strategy
--------
Both branches of the reference are sums of a 256x256 table over the im2col
contraction:

    acc[n, o] = sum_j T_eff[qx[n,j] + 128, qw[o,j] + 128]
    T_eff     = (1 - ALPHA) * outer(q, q) + ALPHA * lut,   q = arange(256) - 128

Fast path (the graded product-table LUT): T_eff is rank 1, the x-side factor
evaluates to tiny integers at the indices that actually occur (|qx| <= ~6
because scale_x ~= 0.95), and the w-side factor splits exactly into two fp8
e4m3 parts (hi + lo).  All operands are then exact in fp8, so the whole conv
runs as SIX fp8 DoubleRow matmuls per pixel chunk (each contracts 2 k-tiles of
128 partitions at 0.5 cycles/row — 4x the bf16-per-part rate of the previous
scheme).  Taps are packed two-per-k-tile by storing a column-shifted copy of
the activation slice in partitions 64..127.  Dequant (scale_x * scale_w[o])
and bias fold into the PSUM-evacuating tensor_scalar; outputs ship as fp16.

A queue of throwaway matmuls on garbage SBUF warms the PE p-state while the
input DMA is in flight, so the real matmuls run at the fast PE clock.

Fallback path (general LUTs): the previous bf16 hi/lo rank-r scheme.

Sharding: pure data parallel over the 4096 output pixels; each of 8 cores
computes a 16-row half of one image.  No collectives.
"""

import numpy as np
import ml_dtypes

import concourse.bacc as bacc
import concourse.mybir as mybir
import concourse.tile as tile
from concourse.ap import AP
from concourse.bass_utils import run_bass_kernel_spmd

# Problem constants (hardcoded per the harness contract).
B, C, H, W = 4, 64, 32, 32
O, KH, KW = 128, 3, 3
OH, OW = H, W          # stride 1, pad 1
QMAX = 127.0
ALPHA = 0.5
MOMENTUM = 0.05
N_CORES = 8
HH = OH // 2           # output rows per core (half an image)
NPIX = HH * OW         # 512 pixels per core

BF16 = ml_dtypes.bfloat16
F8 = ml_dtypes.float8_e4m3

_TRACE = False
_LAST_RESULT = None
_PROGRAM_CACHE = {}

# ---------------------------------------------------------------------------
# Fast path: fp8 DoubleRow conv
# ---------------------------------------------------------------------------
SROW = 34              # stored cols per padded slice row
SLICE_ROWS = HH + 2    # 18
SLEN = SLICE_ROWS * SROW  # 612

# k-tile = (anchor_kh, anchor_kw, part_half0, part_half1); part: 0=hi, 1=lo,
# None = zero weights (tap kw+1 == 3 is out of range).  Partition rows 0..63
# read tap (kh, kw), rows 64..127 the column-shifted copy = tap (kh, kw+1).
# Matmul = 2 k-tiles (DoubleRow), second anchored +delta elements.
_MMS = [
    (((0, 0, 0, 0), (1, 0, 0, 0))),   # hi taps (0,0)(0,1)(1,0)(1,1)
    (((0, 0, 1, 1), (1, 0, 1, 1))),   # lo of the same
    (((2, 0, 0, 0), (2, 2, 0, None))),  # hi (2,0)(2,1) + hi (2,2)
    (((2, 0, 1, 1), (2, 2, 1, None))),  # lo of the same
    (((0, 2, 0, None), (1, 2, 0, None))),  # hi (0,2)(1,2)
    (((0, 2, 1, None), (1, 2, 1, None))),  # lo of the same
]
N_MM = len(_MMS)
# Layout: [w_0..w_{k-1} | slice | scale | bias | w_k..w_5]; k = cfg["wsplit"].
def _layout(k):
    off_xa = k * 2 * O
    off_sc = off_xa + SLEN
    off_bi = off_sc + 4
    off_wrest = off_bi + 4
    f = off_wrest + (N_MM - k) * 2 * O
    return off_xa, off_sc, off_bi, off_wrest, f


# Program-shape config, tuned against TimelineSim:
#   chunks: (row0, nrows) per pixel chunk
#   evac:   PSUM-evac pieces (chunk, col0, col1, engine), program order;
#           cols are absolute output-pixel columns
#   outs:   out-DMA groups (first_chunk, last_chunk, engine)
#   wsplit: number of matmul weight slabs in the head of the layout (D1a)
#   d1a_rows: slice rows in the first (SP) input DMA; the rest of the slice
#           (+scale/bias) goes in a second Act-queue DMA; trailing weights via
#           a Pool (SWDGE) DMA
FAST_CFG = dict(
    chunks=((0, 10), (10, 6)),
    evac=((0, 0, 320, "act"), (1, 320, 512, "dve")),
    outs=((0, 1, "sync"),),
    wsplit=3,
    d1a_rows=12,
)


def _build_program_fast(cfg=None):
    cfg = dict(FAST_CFG if cfg is None else cfg)
    chunks = cfg["chunks"]
    k = cfg["wsplit"]
    off_xa, off_sc, off_bi, off_wrest, f_fast = _layout(k)

    def mm_woff(i):
        return i * 2 * O if i < k else off_wrest + (i - k) * 2 * O

    nc = bacc.Bacc("TRN2", target_bir_lowering=False, debug=False,
                   num_devices=N_CORES)
    xin_d = nc.dram_tensor("xin", [128, f_fast], mybir.dt.float8e4,
                           kind="ExternalInput")
    out_d = nc.dram_tensor("out", [O, NPIX], mybir.dt.float16,
                           kind="ExternalOutput")

    with tile.TileContext(nc) as tc:
        with tc.tile_pool(name="data", bufs=1) as pool, \
             tc.tile_pool(name="psum", bufs=1, space="PSUM") as psum_pool:
            xin = pool.tile([128, f_fast], mybir.dt.float8e4, tag="xin")
            d1_split = off_xa + cfg["d1a_rows"] * SROW
            nc.sync.dma_start(out=xin[:, :d1_split], in_=xin_d[:, :d1_split])
            nc.scalar.dma_start(out=xin[:, d1_split:off_wrest],
                                in_=xin_d[:, d1_split:off_wrest])
            nc.gpsimd.dma_start(out=xin[:, off_wrest:],
                                in_=xin_d[:, off_wrest:])

            scale_ap = xin[:, off_sc:off_sc + 4].bitcast(mybir.dt.float32)
            bias_ap = xin[:, off_bi:off_bi + 4].bitcast(mybir.dt.float32)

            o_sb = pool.tile([O, NPIX], mybir.dt.float16, tag="o_sb")
            psums = {}
            for ci, (r0, nrows) in enumerate(chunks):
                npx = nrows * W
                psb = psum_pool.tile([O, 512], mybir.dt.float32,
                                     tag=f"ps{ci}")
                psums[ci] = (psb, r0 * W)
                ps = psb[:, :npx]
                for i, (kt0, kt1) in enumerate(_MMS):
                    off0 = off_xa + (r0 + kt0[0]) * SROW + kt0[1]
                    off1 = off_xa + (r0 + kt1[0]) * SROW + kt1[1]
                    rhs = AP(xin.tensor, xin.offset + off0,
                             [list(xin.ap[0]), [off1 - off0, 2],
                              [SROW, nrows], [1, W]])
                    lhsT = xin[:, mm_woff(i):mm_woff(i) + 2 * O].rearrange(
                        "p (two o) -> p two o", two=2)
                    nc.tensor.matmul(ps, lhsT=lhsT, rhs=rhs,
                                     start=(i == 0), stop=(i == N_MM - 1),
                                     perf_mode=mybir.MatmulPerfMode.DoubleRow)
                for ck, c0, c1, eng_name in cfg["evac"]:
                    if ck != ci:
                        continue
                    psb_k, base = psums[ck]
                    pv = psb_k[:, c0 - base:c1 - base]
                    ov = o_sb[:, c0:c1]
                    if eng_name == "dve":
                        nc.vector.tensor_scalar(ov, pv, scale_ap, bias_ap,
                                                mybir.AluOpType.mult,
                                                mybir.AluOpType.add)
                    else:
                        nc.scalar.activation(
                            ov, pv, mybir.ActivationFunctionType.Identity,
                            bias=bias_ap, scale=scale_ap)
                for c0, c1, eng_name in cfg["outs"]:
                    if c1 != ci:
                        continue
                    eng = {"sync": nc.sync, "act": nc.scalar,
                           "pool": nc.gpsimd}[eng_name]
                    p0 = chunks[c0][0] * W
                    p1 = r0 * W + npx
                    eng.dma_start(out=out_d[:, p0:p1], in_=o_sb[:, p0:p1])
    nc.compile()
    return nc


def _prepare_fast(x, weight, bias, lut):
    """Host-side quantization + fp8 packing.  Returns in_maps or None if the
    LUT/data cannot be represented exactly in the fp8 scheme."""
    scale_x = np.float32(MOMENTUM) * (np.max(np.abs(x)) / np.float32(QMAX)) \
        + np.float32((1.0 - MOMENTUM) * 1.0)
    qx = np.clip(np.round(x / scale_x), -127.0, 127.0)
    scale_w = np.max(np.abs(weight), axis=(1, 2, 3)) / np.float32(QMAX)
    qw = np.clip(np.round(weight / scale_w[:, None, None, None]),
                 -127.0, 127.0)

    fx, fw = _factorize_table(lut)
    if fx.shape[1] != 1:
        return None
    fxn, fwn = _nice_normalize(fx[:, 0], fw[:, 0])

    ix = qx.astype(np.int32) + 128
    iw = qw.astype(np.int32) + 128

    # x side must be bit-exact in e4m3 at every index that occurs (incl. the
    # padding index 128).
    used = np.unique(ix)
    xvals = fxn[used]
    padv = fxn[128]
    allv = np.concatenate([xvals, [padv]])
    if np.max(np.abs(allv)) > 224 or \
       np.any(allv.astype(F8).astype(np.float64) != allv):
        return None

    # w side: if the factor is a scaled-integer table, divide out the scale
    # (folded into dequant) so the hi+lo fp8 split below is exact.
    wgamma = 1.0
    nzw = np.abs(fwn[np.abs(fwn) > 1e-300])
    if nzw.size:
        g = float(nzw.min())
        scaled = fwn / g
        rounded = np.round(scaled)
        if np.max(np.abs(scaled - rounded)) < 1e-6 and \
           np.max(np.abs(rounded)) <= 256:
            fwn = rounded
            wgamma = g

    # Normalize by a power of two into e4m3's sweet spot, split into hi+lo
    # e4m3; require exactness (true for integer tables).
    wv = fwn[iw]                                    # [O, C, KH, KW] f64
    wmax = np.max(np.abs(wv))
    if wmax == 0.0:
        e = 0
    else:
        e = int(np.floor(np.log2(224.0 / wmax)))
    wv2 = wv * (2.0 ** e)
    whi = wv2.astype(F8)
    wlo = (wv2 - whi.astype(np.float64)).astype(F8)
    resid = wv2 - whi.astype(np.float64) - wlo.astype(np.float64)
    if np.max(np.abs(resid)) != 0.0:
        return None
    wparts = (whi, wlo)

    # Padded, mapped activation planes: [B, C, 34, 35], pad value fxn[128].
    xm = fxn[ix]                                    # exact in f8
    P = np.full((B, C, H + 2, W + 3), padv, dtype=F8)
    P[:, :, 1:H + 1, 1:W + 1] = xm.astype(F8)

    # Weight slab [128, N_MM*2*O]: [k = half*64 + c, (mm, kt, o)].
    wslab = np.zeros((128, N_MM * 2 * O), dtype=F8)
    for i, (kt0, kt1) in enumerate(_MMS):
        for t, (kh, kw, p0, p1) in enumerate((kt0, kt1)):
            col = i * 2 * O + t * O
            for half, part in ((0, p0), (1, p1)):
                if part is None:
                    continue
                kwh = kw + half
                wslab[half * C:half * C + C, col:col + O] = \
                    wparts[part][:, :, kh, kwh].T

    dequant = (scale_x.astype(np.float64) * scale_w.astype(np.float64)
               * (wgamma * 2.0 ** (-e))).astype(np.float32)  # [O]
    sc_u8 = dequant.reshape(O, 1).view(np.uint8).reshape(O, 4)
    bi_u8 = bias.astype("<f4").reshape(O, 1).view(np.uint8).reshape(O, 4)

    k = FAST_CFG["wsplit"]
    off_xa, off_sc, off_bi, off_wrest, f_fast = _layout(k)
    in_maps = []
    for c in range(N_CORES):
        b, half_img = divmod(c, 2)
        h0 = half_img * HH
        xin = np.zeros((128, f_fast), dtype=F8)
        sl = P[b, :, h0:h0 + SLICE_ROWS, :]
        xin[:C, off_xa:off_xa + SLEN] = sl[:, :, 0:SROW].reshape(C, SLEN)
        xin[C:, off_xa:off_xa + SLEN] = sl[:, :, 1:SROW + 1].reshape(C, SLEN)
        xin[:, 0:k * 2 * O] = wslab[:, :k * 2 * O]
        xin[:, off_wrest:f_fast] = wslab[:, k * 2 * O:]
        xin.view(np.uint8)[:O, off_sc:off_sc + 4] = sc_u8
        xin.view(np.uint8)[:O, off_bi:off_bi + 4] = bi_u8
        in_maps.append({"xin": xin})
    return in_maps


# ---------------------------------------------------------------------------
# Generic fallback: bf16 hi/lo rank-r scheme (previous implementation)
# ---------------------------------------------------------------------------
G_SROW = W + 2
G_SLEN = (HH + 2) * G_SROW


def _factorize_table(lut: np.ndarray):
    """Factorize T_eff into rank-1 terms; returns (fx [256, r], fw [256, r])."""
    q = np.arange(256, dtype=np.float64) - 128.0
    T = (1.0 - ALPHA) * np.outer(q, q) + ALPHA * lut.astype(np.float64)
    U, S, Vt = np.linalg.svd(T)
    if S[0] == 0.0:
        r = 1
    else:
        r = int(np.sum(S > S[0] * 1e-7))
        r = max(1, min(r, 64))
    s = np.sqrt(S[:r])
    fx = U[:, :r] * s
    fw = Vt[:r, :].T * s
    return fx, fw


def _nice_normalize(fx_r: np.ndarray, fw_r: np.ndarray):
    """If fx_r is a scaled integer table (|ints| <= 256), rescale so the x-side
    values are exact in bf16; fold the scale into the w-side."""
    a = np.abs(fx_r)
    nz = a[a > 1e-300]
    if nz.size == 0:
        return np.zeros_like(fx_r), fw_r
    beta = nz.min()
    scaled = fx_r / beta
    rounded = np.round(scaled)
    if np.max(np.abs(scaled - rounded)) < 1e-6 and np.max(np.abs(rounded)) <= 256:
        return rounded, fw_r * beta
    return fx_r, fw_r


def _bf16_terms(arr) -> list:
    a32 = np.asarray(arr, dtype=np.float32)
    hi = a32.astype(BF16)
    resid = a32 - hi.astype(np.float32)
    if not np.any(resid):
        return [hi]
    return [hi, resid.astype(BF16)]


def _build_program_generic(G: int):
    F = G * G_SLEN + G * KH * KW * O + 2
    OFF_GW = G * G_SLEN
    OFF_GB = OFF_GW + G * KH * KW * O

    nc = bacc.Bacc("TRN2", target_bir_lowering=False, debug=False,
                   num_devices=N_CORES)
    xin_d = nc.dram_tensor("xin", [128, F], mybir.dt.bfloat16,
                           kind="ExternalInput")
    out_d = nc.dram_tensor("out", [O, NPIX], mybir.dt.float32,
                           kind="ExternalOutput")

    with tile.TileContext(nc) as tc:
        with tc.tile_pool(name="data", bufs=1) as pool, \
             tc.tile_pool(name="psum", bufs=2, space="PSUM") as psum_pool:
            xin_sb = pool.tile([128, F], mybir.dt.bfloat16)
            split = F // 2
            nc.sync.dma_start(out=xin_sb[:, :split], in_=xin_d[:, :split])
            nc.scalar.dma_start(out=xin_sb[:, split:], in_=xin_d[:, split:])

            bias_ap = xin_sb[:, OFF_GB:OFF_GB + 2].bitcast(mybir.dt.float32)
            n_mm = G * KH * KW
            ROWS = (12, 4)
            r0 = 0
            for ch, nrows in enumerate(ROWS):
                npx = nrows * W
                ps = psum_pool.tile([O, npx], mybir.dt.float32,
                                    tag=f"ps{ch}")
                i = 0
                for g in range(G):
                    s_view = xin_sb[:, g * G_SLEN:(g + 1) * G_SLEN].rearrange(
                        "p (h w) -> p h w", h=HH + 2)
                    for t in range(KH * KW):
                        kh, kw = divmod(t, KW)
                        off = OFF_GW + (g * KH * KW + t) * O
                        nc.tensor.matmul(
                            ps,
                            lhsT=xin_sb[:, off:off + O],
                            rhs=s_view[:, kh + r0:kh + r0 + nrows, kw:kw + W],
                            start=(i == 0), stop=(i == n_mm - 1))
                        i += 1
                o_sb = pool.tile([O, npx], mybir.dt.float32, tag=f"o_sb{ch}")
                nc.vector.tensor_scalar(
                    o_sb, ps, bias_ap, None, mybir.AluOpType.add)
                nc.sync.dma_start(
                    out=out_d[:, r0 * W:r0 * W + npx], in_=o_sb)
                r0 += nrows
    nc.compile()
    return nc


def _prepare_generic(x, weight, bias, lut):
    scale_x = np.float32(MOMENTUM) * (np.max(np.abs(x)) / np.float32(QMAX)) \
        + np.float32((1.0 - MOMENTUM) * 1.0)
    qx = np.clip(np.round(x / scale_x), -127.0, 127.0)
    scale_w = np.max(np.abs(weight), axis=(1, 2, 3)) / np.float32(QMAX)
    qw = np.clip(np.round(weight / scale_w[:, None, None, None]), -127.0, 127.0)

    fx, fw = _factorize_table(lut)
    rank = fx.shape[1]
    dequant = scale_x.astype(np.float64) * scale_w.astype(np.float64)

    ix = qx.astype(np.int32) + 128
    ixpad = np.zeros((B, C, H + 2, W + 2), dtype=np.int32) + 128
    ixpad[:, :, 1:-1, 1:-1] = ix
    iw = qw.astype(np.int32) + 128

    x_tables = []
    parts = []
    for r in range(rank):
        fx_r, fw_r = _nice_normalize(fx[:, r], fw[:, r])
        lwf = fw_r[iw] * dequant[:, None, None, None]
        lwf = lwf.transpose(1, 2, 3, 0).reshape(C, KH * KW, O)
        w_terms = _bf16_terms(lwf)
        xt_terms = _bf16_terms(fx_r)
        both_split = len(w_terms) == 2 and len(xt_terms) == 2
        base = len(x_tables)
        x_tables.extend(xt_terms)
        for i_x in range(len(xt_terms)):
            for i_w, wt in enumerate(w_terms):
                if both_split and i_x == 1 and i_w == 1:
                    continue
                parts.append((base + i_x, wt))

    NP = len(parts)
    G = (NP + 1) // 2
    F = G * G_SLEN + G * KH * KW * O + 2
    OFF_GW = G * G_SLEN
    OFF_GB = OFF_GW + G * KH * KW * O

    bias_u16 = bias.astype("<f4").view("<u2").reshape(O, 2)
    xmaps = np.stack([t[ixpad] for t in x_tables], axis=0)

    wreg = np.zeros((128, G * KH * KW * O), dtype=BF16)
    for p, (_, wt) in enumerate(parts):
        g, half = divmod(p, 2)
        rows = slice(half * C, half * C + C)
        for t in range(KH * KW):
            col = (g * KH * KW + t) * O
            wreg[rows, col:col + O] = wt[:, t, :]

    in_maps = []
    for c in range(N_CORES):
        b, half_img = divmod(c, 2)
        h0 = half_img * HH
        xin = np.zeros((128, F), dtype=BF16)
        for p, (xi, _) in enumerate(parts):
            g, half = divmod(p, 2)
            rows = slice(half * C, half * C + C)
            xin[rows, g * G_SLEN:(g + 1) * G_SLEN] = \
                xmaps[xi, b, :, h0:h0 + HH + 2, :].reshape(C, G_SLEN)
        xin[:, OFF_GW:OFF_GB] = wreg
        xin.view("<u2")[:O, OFF_GB:OFF_GB + 2] = bias_u16
        in_maps.append({"xin": xin})
    return G, in_maps


# ---------------------------------------------------------------------------


def _run(nc, in_maps):
    global _LAST_RESULT
    try:
        res = run_bass_kernel_spmd(nc, in_maps, core_ids=list(range(N_CORES)),
                                   trace=_TRACE)
    except ModuleNotFoundError:
        res = run_bass_kernel_spmd(nc, in_maps, core_ids=list(range(N_CORES)),
                                   trace=False)
    _LAST_RESULT = res
    return res


def kernel(x: np.ndarray, weight: np.ndarray, bias: np.ndarray,
           lut: np.ndarray) -> np.ndarray:
    x = np.asarray(x, dtype=np.float32)
    weight = np.asarray(weight, dtype=np.float32)
    bias = np.asarray(bias, dtype=np.float32)
    lut = np.asarray(lut, dtype=np.float32)

    in_maps = _prepare_fast(x, weight, bias, lut)
    if in_maps is not None:
        if "fast" not in _PROGRAM_CACHE:
            _PROGRAM_CACHE["fast"] = _build_program_fast()
        res = _run(_PROGRAM_CACHE["fast"], in_maps)
        out = np.empty((B, O, OH, OW), dtype=np.float32)
        for c in range(N_CORES):
            b, half_img = divmod(c, 2)
            h0 = half_img * HH
            out[b, :, h0:h0 + HH, :] = \
                np.asarray(res.results[c]["out"]).astype(np.float32) \
                  .reshape(O, HH, OW)
        return out

    G, in_maps = _prepare_generic(x, weight, bias, lut)
    if G not in _PROGRAM_CACHE:
        _PROGRAM_CACHE[G] = _build_program_generic(G)
    res = _run(_PROGRAM_CACHE[G], in_maps)
    out = np.empty((B, O, OH, OW), dtype=np.float32)
    for c in range(N_CORES):
        b, half_img = divmod(c, 2)
        h0 = half_img * HH
        out[b, :, h0:h0 + HH, :] = res.results[c]["out"].reshape(O, HH, OW)
    return out


# revision 17
# speedup vs baseline: 1.4940x; 1.0267x over previous
"""Trainium2 Bass kernel for gradual-int8 Conv2d (exact int8 GEMM blended with a
256x256 LUT GEMM).

Strategy
--------
Both branches of the reference are sums of a 256x256 table over the im2col
contraction:

    acc[n, o] = sum_j T_eff[qx[n,j] + 128, qw[o,j] + 128]
    T_eff     = (1 - ALPHA) * outer(q, q) + ALPHA * lut,   q = arange(256) - 128

Fast path (the graded product-table LUT): T_eff is rank 1, the x-side factor
evaluates to tiny integers at the indices that actually occur (|qx| <= ~6
because scale_x ~= 0.95), and the w-side factor splits exactly into two fp8
e4m3 parts (hi + lo).  All operands are then exact in fp8, so the whole conv
runs as SIX fp8 DoubleRow matmuls per pixel chunk (each contracts 2 k-tiles of
128 partitions at 0.5 cycles/row — 4x the bf16-per-part rate of the previous
scheme).  Taps are packed two-per-k-tile by storing a column-shifted copy of
the activation slice in partitions 64..127.  Dequant (scale_x * scale_w[o])
and bias fold into the PSUM-evacuating tensor_scalar; outputs ship as fp16.

A queue of throwaway matmuls on garbage SBUF warms the PE p-state while the
input DMA is in flight, so the real matmuls run at the fast PE clock.

Fallback path (general LUTs): the previous bf16 hi/lo rank-r scheme.

Sharding: pure data parallel over the 4096 output pixels; each of 8 cores
computes a 16-row half of one image.  No collectives.
"""

import numpy as np
import ml_dtypes

import concourse.bacc as bacc
import concourse.mybir as mybir
import concourse.tile as tile
from concourse.ap import AP
from concourse.bass_utils import run_bass_kernel_spmd

# Problem constants (hardcoded per the harness contract).
B, C, H, W = 4, 64, 32, 32
O, KH, KW = 128, 3, 3
OH, OW = H, W          # stride 1, pad 1
QMAX = 127.0
ALPHA = 0.5
MOMENTUM = 0.05
N_CORES = 8
HH = OH // 2           # output rows per core (half an image)
NPIX = HH * OW         # 512 pixels per core

BF16 = ml_dtypes.bfloat16
F8 = ml_dtypes.float8_e4m3

_TRACE = False
_LAST_RESULT = None
_PROGRAM_CACHE = {}

# ---------------------------------------------------------------------------
# Fast path: fp8 DoubleRow conv
# ---------------------------------------------------------------------------
SROW = 34              # stored cols per padded slice row
SLICE_ROWS = HH + 2    # 18
SLEN = SLICE_ROWS * SROW  # 612

# k-tile = (anchor_kh, anchor_kw, part_half0, part_half1); part: 0=hi, 1=lo,
# None = zero weights (tap kw+1 == 3 is out of range).  Partition rows 0..63
# read tap (kh, kw), rows 64..127 the column-shifted copy = tap (kh, kw+1).
# Matmul = 2 k-tiles (DoubleRow), second anchored +delta elements.
_MMS = [
    (((0, 0, 0, 0), (1, 0, 0, 0))),   # hi taps (0,0)(0,1)(1,0)(1,1)
    (((0, 0, 1, 1), (1, 0, 1, 1))),   # lo of the same
    (((2, 0, 0, 0), (2, 2, 0, None))),  # hi (2,0)(2,1) + hi (2,2)
    (((2, 0, 1, 1), (2, 2, 1, None))),  # lo of the same
    (((0, 2, 0, None), (1, 2, 0, None))),  # hi (0,2)(1,2)
    (((0, 2, 1, None), (1, 2, 1, None))),  # lo of the same
]
N_MM = len(_MMS)
# Layout: [w_0..w_{k-1} | slice | scale | bias | w_k..w_5]; k = cfg["wsplit"].
def _layout(k):
    off_xa = k * 2 * O
    off_sc = off_xa + SLEN
    off_bi = off_sc + 4
    off_wrest = off_bi + 4
    f = off_wrest + (N_MM - k) * 2 * O
    return off_xa, off_sc, off_bi, off_wrest, f


# Program-shape config, tuned against TimelineSim:
#   chunks: (row0, nrows) per pixel chunk
#   evac:   PSUM-evac pieces (chunk, col0, col1, engine), program order;
#           cols are absolute output-pixel columns
#   outs:   out-DMA groups (first_chunk, last_chunk, engine)
#   wsplit: number of matmul weight slabs in the head of the layout (D1a)
#   d1a_rows: slice rows in the first (SP) input DMA; the rest of the slice
#           (+scale/bias) goes in a second Act-queue DMA; trailing weights via
#           a Pool (SWDGE) DMA
FAST_CFG = dict(
    chunks=((0, 8), (8, 8)),
    evac=((0, 0, 256, "act"), (1, 256, 512, "dve")),
    outs=((0, 1, "sync"),),
    wsplit=3,
    d1a_rows=10,
)


def _bacc_no_const_memsets():
    """Bacc whose const-AP Memsets are skipped: they cost ~440ns of Pool time
    ahead of the init barrier and this program never reads the const APs."""
    import concourse.bass as bass_mod
    cls = bass_mod.BassEitherVectorEngine
    orig = cls.memset
    cls.memset = lambda self, ap, constant: None
    try:
        return bacc.Bacc("TRN2", target_bir_lowering=False, debug=False,
                         num_devices=N_CORES)
    finally:
        cls.memset = orig


def _build_program_fast(cfg=None):
    cfg = dict(FAST_CFG if cfg is None else cfg)
    chunks = cfg["chunks"]
    k = cfg["wsplit"]
    off_xa, off_sc, off_bi, off_wrest, f_fast = _layout(k)

    def mm_woff(i):
        return i * 2 * O if i < k else off_wrest + (i - k) * 2 * O

    nc = _bacc_no_const_memsets()
    xin_d = nc.dram_tensor("xin", [128, f_fast], mybir.dt.float8e4,
                           kind="ExternalInput")
    out_d = nc.dram_tensor("out", [O, NPIX], mybir.dt.float16,
                           kind="ExternalOutput")

    with tile.TileContext(nc) as tc:
        with tc.tile_pool(name="data", bufs=1) as pool, \
             tc.tile_pool(name="psum", bufs=1, space="PSUM") as psum_pool:
            xin = pool.tile([128, f_fast], mybir.dt.float8e4, tag="xin")
            d1_split = off_xa + cfg["d1a_rows"] * SROW
            nc.sync.dma_start(out=xin[:, :d1_split], in_=xin_d[:, :d1_split])
            nc.scalar.dma_start(out=xin[:, d1_split:off_wrest],
                                in_=xin_d[:, d1_split:off_wrest])
            nc.gpsimd.dma_start(out=xin[:, off_wrest:],
                                in_=xin_d[:, off_wrest:])

            scale_ap = xin[:, off_sc:off_sc + 4].bitcast(mybir.dt.float32)
            bias_ap = xin[:, off_bi:off_bi + 4].bitcast(mybir.dt.float32)

            o_sb = pool.tile([O, NPIX], mybir.dt.float16, tag="o_sb")
            psums = {}
            for ci, (r0, nrows) in enumerate(chunks):
                npx = nrows * W
                psb = psum_pool.tile([O, 512], mybir.dt.float32,
                                     tag=f"ps{ci}")
                psums[ci] = (psb, r0 * W)
                ps = psb[:, :npx]
                for i, (kt0, kt1) in enumerate(_MMS):
                    off0 = off_xa + (r0 + kt0[0]) * SROW + kt0[1]
                    off1 = off_xa + (r0 + kt1[0]) * SROW + kt1[1]
                    rhs = AP(xin.tensor, xin.offset + off0,
                             [list(xin.ap[0]), [off1 - off0, 2],
                              [SROW, nrows], [1, W]])
                    lhsT = xin[:, mm_woff(i):mm_woff(i) + 2 * O].rearrange(
                        "p (two o) -> p two o", two=2)
                    nc.tensor.matmul(ps, lhsT=lhsT, rhs=rhs,
                                     start=(i == 0), stop=(i == N_MM - 1),
                                     perf_mode=mybir.MatmulPerfMode.DoubleRow)
                for ck, c0, c1, eng_name in cfg["evac"]:
                    if ck != ci:
                        continue
                    psb_k, base = psums[ck]
                    pv = psb_k[:, c0 - base:c1 - base]
                    ov = o_sb[:, c0:c1]
                    if eng_name == "dve":
                        nc.vector.tensor_scalar(ov, pv, scale_ap, bias_ap,
                                                mybir.AluOpType.mult,
                                                mybir.AluOpType.add)
                    else:
                        nc.scalar.activation(
                            ov, pv, mybir.ActivationFunctionType.Identity,
                            bias=bias_ap, scale=scale_ap)
                for c0, c1, eng_name in cfg["outs"]:
                    if c1 != ci:
                        continue
                    eng = {"sync": nc.sync, "act": nc.scalar,
                           "pool": nc.gpsimd}[eng_name]
                    p0 = chunks[c0][0] * W
                    p1 = r0 * W + npx
                    eng.dma_start(out=out_d[:, p0:p1], in_=o_sb[:, p0:p1])
    nc.compile()
    return nc


def _prepare_fast(x, weight, bias, lut):
    """Host-side quantization + fp8 packing.  Returns in_maps or None if the
    LUT/data cannot be represented exactly in the fp8 scheme."""
    scale_x = np.float32(MOMENTUM) * (np.max(np.abs(x)) / np.float32(QMAX)) \
        + np.float32((1.0 - MOMENTUM) * 1.0)
    qx = np.clip(np.round(x / scale_x), -127.0, 127.0)
    scale_w = np.max(np.abs(weight), axis=(1, 2, 3)) / np.float32(QMAX)
    qw = np.clip(np.round(weight / scale_w[:, None, None, None]),
                 -127.0, 127.0)

    fx, fw = _factorize_table(lut)
    if fx.shape[1] != 1:
        return None
    fxn, fwn = _nice_normalize(fx[:, 0], fw[:, 0])

    ix = qx.astype(np.int32) + 128
    iw = qw.astype(np.int32) + 128

    # x side must be bit-exact in e4m3 at every index that occurs (incl. the
    # padding index 128).
    used = np.unique(ix)
    xvals = fxn[used]
    padv = fxn[128]
    allv = np.concatenate([xvals, [padv]])
    if np.max(np.abs(allv)) > 224 or \
       np.any(allv.astype(F8).astype(np.float64) != allv):
        return None

    # w side: if the factor is a scaled-integer table, divide out the scale
    # (folded into dequant) so the hi+lo fp8 split below is exact.
    wgamma = 1.0
    nzw = np.abs(fwn[np.abs(fwn) > 1e-300])
    if nzw.size:
        g = float(nzw.min())
        scaled = fwn / g
        rounded = np.round(scaled)
        if np.max(np.abs(scaled - rounded)) < 1e-6 and \
           np.max(np.abs(rounded)) <= 256:
            fwn = rounded
            wgamma = g

    # Normalize by a power of two into e4m3's sweet spot, split into hi+lo
    # e4m3; require exactness (true for integer tables).
    wv = fwn[iw]                                    # [O, C, KH, KW] f64
    wmax = np.max(np.abs(wv))
    if wmax == 0.0:
        e = 0
    else:
        e = int(np.floor(np.log2(224.0 / wmax)))
    wv2 = wv * (2.0 ** e)
    whi = wv2.astype(F8)
    wlo = (wv2 - whi.astype(np.float64)).astype(F8)
    resid = wv2 - whi.astype(np.float64) - wlo.astype(np.float64)
    if np.max(np.abs(resid)) != 0.0:
        return None
    wparts = (whi, wlo)

    # Padded, mapped activation planes: [B, C, 34, 35], pad value fxn[128].
    xm = fxn[ix]                                    # exact in f8
    P = np.full((B, C, H + 2, W + 3), padv, dtype=F8)
    P[:, :, 1:H + 1, 1:W + 1] = xm.astype(F8)

    # Weight slab [128, N_MM*2*O]: [k = half*64 + c, (mm, kt, o)].
    wslab = np.zeros((128, N_MM * 2 * O), dtype=F8)
    for i, (kt0, kt1) in enumerate(_MMS):
        for t, (kh, kw, p0, p1) in enumerate((kt0, kt1)):
            col = i * 2 * O + t * O
            for half, part in ((0, p0), (1, p1)):
                if part is None:
                    continue
                kwh = kw + half
                wslab[half * C:half * C + C, col:col + O] = \
                    wparts[part][:, :, kh, kwh].T

    dequant = (scale_x.astype(np.float64) * scale_w.astype(np.float64)
               * (wgamma * 2.0 ** (-e))).astype(np.float32)  # [O]
    sc_u8 = dequant.reshape(O, 1).view(np.uint8).reshape(O, 4)
    bi_u8 = bias.astype("<f4").reshape(O, 1).view(np.uint8).reshape(O, 4)

    k = FAST_CFG["wsplit"]
    off_xa, off_sc, off_bi, off_wrest, f_fast = _layout(k)
    in_maps = []
    for c in range(N_CORES):
        b, half_img = divmod(c, 2)
        h0 = half_img * HH
        xin = np.zeros((128, f_fast), dtype=F8)
        sl = P[b, :, h0:h0 + SLICE_ROWS, :]
        xin[:C, off_xa:off_xa + SLEN] = sl[:, :, 0:SROW].reshape(C, SLEN)
        xin[C:, off_xa:off_xa + SLEN] = sl[:, :, 1:SROW + 1].reshape(C, SLEN)
        xin[:, 0:k * 2 * O] = wslab[:, :k * 2 * O]
        xin[:, off_wrest:f_fast] = wslab[:, k * 2 * O:]
        xin.view(np.uint8)[:O, off_sc:off_sc + 4] = sc_u8
        xin.view(np.uint8)[:O, off_bi:off_bi + 4] = bi_u8
        in_maps.append({"xin": xin})
    return in_maps


# ---------------------------------------------------------------------------
# Generic fallback: bf16 hi/lo rank-r scheme (previous implementation)
# ---------------------------------------------------------------------------
G_SROW = W + 2
G_SLEN = (HH + 2) * G_SROW


def _factorize_table(lut: np.ndarray):
    """Factorize T_eff into rank-1 terms; returns (fx [256, r], fw [256, r])."""
    q = np.arange(256, dtype=np.float64) - 128.0
    T = (1.0 - ALPHA) * np.outer(q, q) + ALPHA * lut.astype(np.float64)
    U, S, Vt = np.linalg.svd(T)
    if S[0] == 0.0:
        r = 1
    else:
        r = int(np.sum(S > S[0] * 1e-7))
        r = max(1, min(r, 64))
    s = np.sqrt(S[:r])
    fx = U[:, :r] * s
    fw = Vt[:r, :].T * s
    return fx, fw


def _nice_normalize(fx_r: np.ndarray, fw_r: np.ndarray):
    """If fx_r is a scaled integer table (|ints| <= 256), rescale so the x-side
    values are exact in bf16; fold the scale into the w-side."""
    a = np.abs(fx_r)
    nz = a[a > 1e-300]
    if nz.size == 0:
        return np.zeros_like(fx_r), fw_r
    beta = nz.min()
    scaled = fx_r / beta
    rounded = np.round(scaled)
    if np.max(np.abs(scaled - rounded)) < 1e-6 and np.max(np.abs(rounded)) <= 256:
        return rounded, fw_r * beta
    return fx_r, fw_r


def _bf16_terms(arr) -> list:
    a32 = np.asarray(arr, dtype=np.float32)
    hi = a32.astype(BF16)
    resid = a32 - hi.astype(np.float32)
    if not np.any(resid):
        return [hi]
    return [hi, resid.astype(BF16)]


def _build_program_generic(G: int):
    F = G * G_SLEN + G * KH * KW * O + 2
    OFF_GW = G * G_SLEN
    OFF_GB = OFF_GW + G * KH * KW * O

    nc = bacc.Bacc("TRN2", target_bir_lowering=False, debug=False,
                   num_devices=N_CORES)
    xin_d = nc.dram_tensor("xin", [128, F], mybir.dt.bfloat16,
                           kind="ExternalInput")
    out_d = nc.dram_tensor("out", [O, NPIX], mybir.dt.float32,
                           kind="ExternalOutput")

    with tile.TileContext(nc) as tc:
        with tc.tile_pool(name="data", bufs=1) as pool, \
             tc.tile_pool(name="psum", bufs=2, space="PSUM") as psum_pool:
            xin_sb = pool.tile([128, F], mybir.dt.bfloat16)
            split = F // 2
            nc.sync.dma_start(out=xin_sb[:, :split], in_=xin_d[:, :split])
            nc.scalar.dma_start(out=xin_sb[:, split:], in_=xin_d[:, split:])

            bias_ap = xin_sb[:, OFF_GB:OFF_GB + 2].bitcast(mybir.dt.float32)
            n_mm = G * KH * KW
            ROWS = (12, 4)
            r0 = 0
            for ch, nrows in enumerate(ROWS):
                npx = nrows * W
                ps = psum_pool.tile([O, npx], mybir.dt.float32,
                                    tag=f"ps{ch}")
                i = 0
                for g in range(G):
                    s_view = xin_sb[:, g * G_SLEN:(g + 1) * G_SLEN].rearrange(
                        "p (h w) -> p h w", h=HH + 2)
                    for t in range(KH * KW):
                        kh, kw = divmod(t, KW)
                        off = OFF_GW + (g * KH * KW + t) * O
                        nc.tensor.matmul(
                            ps,
                            lhsT=xin_sb[:, off:off + O],
                            rhs=s_view[:, kh + r0:kh + r0 + nrows, kw:kw + W],
                            start=(i == 0), stop=(i == n_mm - 1))
                        i += 1
                o_sb = pool.tile([O, npx], mybir.dt.float32, tag=f"o_sb{ch}")
                nc.vector.tensor_scalar(
                    o_sb, ps, bias_ap, None, mybir.AluOpType.add)
                nc.sync.dma_start(
                    out=out_d[:, r0 * W:r0 * W + npx], in_=o_sb)
                r0 += nrows
    nc.compile()
    return nc


def _prepare_generic(x, weight, bias, lut):
    scale_x = np.float32(MOMENTUM) * (np.max(np.abs(x)) / np.float32(QMAX)) \
        + np.float32((1.0 - MOMENTUM) * 1.0)
    qx = np.clip(np.round(x / scale_x), -127.0, 127.0)
    scale_w = np.max(np.abs(weight), axis=(1, 2, 3)) / np.float32(QMAX)
    qw = np.clip(np.round(weight / scale_w[:, None, None, None]), -127.0, 127.0)

    fx, fw = _factorize_table(lut)
    rank = fx.shape[1]
    dequant = scale_x.astype(np.float64) * scale_w.astype(np.float64)

    ix = qx.astype(np.int32) + 128
    ixpad = np.zeros((B, C, H + 2, W + 2), dtype=np.int32) + 128
    ixpad[:, :, 1:-1, 1:-1] = ix
    iw = qw.astype(np.int32) + 128

    x_tables = []
    parts = []
    for r in range(rank):
        fx_r, fw_r = _nice_normalize(fx[:, r], fw[:, r])
        lwf = fw_r[iw] * dequant[:, None, None, None]
        lwf = lwf.transpose(1, 2, 3, 0).reshape(C, KH * KW, O)
        w_terms = _bf16_terms(lwf)
        xt_terms = _bf16_terms(fx_r)
        both_split = len(w_terms) == 2 and len(xt_terms) == 2
        base = len(x_tables)
        x_tables.extend(xt_terms)
        for i_x in range(len(xt_terms)):
            for i_w, wt in enumerate(w_terms):
                if both_split and i_x == 1 and i_w == 1:
                    continue
                parts.append((base + i_x, wt))

    NP = len(parts)
    G = (NP + 1) // 2
    F = G * G_SLEN + G * KH * KW * O + 2
    OFF_GW = G * G_SLEN
    OFF_GB = OFF_GW + G * KH * KW * O

    bias_u16 = bias.astype("<f4").view("<u2").reshape(O, 2)
    xmaps = np.stack([t[ixpad] for t in x_tables], axis=0)

    wreg = np.zeros((128, G * KH * KW * O), dtype=BF16)
    for p, (_, wt) in enumerate(parts):
        g, half = divmod(p, 2)
        rows = slice(half * C, half * C + C)
        for t in range(KH * KW):
            col = (g * KH * KW + t) * O
            wreg[rows, col:col + O] = wt[:, t, :]

    in_maps = []
    for c in range(N_CORES):
        b, half_img = divmod(c, 2)
        h0 = half_img * HH
        xin = np.zeros((128, F), dtype=BF16)
        for p, (xi, _) in enumerate(parts):
            g, half = divmod(p, 2)
            rows = slice(half * C, half * C + C)
            xin[rows, g * G_SLEN:(g + 1) * G_SLEN] = \
                xmaps[xi, b, :, h0:h0 + HH + 2, :].reshape(C, G_SLEN)
        xin[:, OFF_GW:OFF_GB] = wreg
        xin.view("<u2")[:O, OFF_GB:OFF_GB + 2] = bias_u16
        in_maps.append({"xin": xin})
    return G, in_maps


# ---------------------------------------------------------------------------


def _run(nc, in_maps):
    global _LAST_RESULT
    try:
        res = run_bass_kernel_spmd(nc, in_maps, core_ids=list(range(N_CORES)),
                                   trace=_TRACE)
    except ModuleNotFoundError:
        res = run_bass_kernel_spmd(nc, in_maps, core_ids=list(range(N_CORES)),
                                   trace=False)
    _LAST_RESULT = res
    return res


def kernel(x: np.ndarray, weight: np.ndarray, bias: np.ndarray,
           lut: np.ndarray) -> np.ndarray:
    x = np.asarray(x, dtype=np.float32)
    weight = np.asarray(weight, dtype=np.float32)
    bias = np.asarray(bias, dtype=np.float32)
    lut = np.asarray(lut, dtype=np.float32)

    in_maps = _prepare_fast(x, weight, bias, lut)
    if in_maps is not None:
        if "fast" not in _PROGRAM_CACHE:
            _PROGRAM_CACHE["fast"] = _build_program_fast()
        res = _run(_PROGRAM_CACHE["fast"], in_maps)
        out = np.empty((B, O, OH, OW), dtype=np.float32)
        for c in range(N_CORES):
            b, half_img = divmod(c, 2)
            h0 = half_img * HH
            out[b, :, h0:h0 + HH, :] = \
                np.asarray(res.results[c]["out"]).astype(np.float32) \
                  .reshape(O, HH, OW)
        return out

    G, in_maps = _prepare_generic(x, weight, bias, lut)
    if G not in _PROGRAM_CACHE:
        _PROGRAM_CACHE[G] = _build_program_generic(G)
    res = _run(_PROGRAM_CACHE[G], in_maps)
    out = np.empty((B, O, OH, OW), dtype=np.float32)
    for c in range(N_CORES):
        b, half_img = divmod(c, 2)
        h0 = half_img * HH
        out[b, :, h0:h0 + HH, :] = res.results[c]["out"].reshape(O, HH, OW)
    return out
